# revision 1
# baseline (speedup 1.0000x reference)
"""Trainium2 Bass kernel for IrrepWiseLinear.

out[n, m, :] = x[n, m, :] @ weight[seg_id(m)]   (seg sizes [1,3,5,7], DIM=16)

Strategy: data-parallel over the 8 NeuronCores on the leading N dim.
Per core: stream x in 1MB blocks of 128 nodes ([128, 16*128] f32, 8KB
contiguous per partition line), PE-transpose each per-m [128n, 128c] slice
(fp32 transpose mode), copy PSUM->SBUF, then fp32 matmul with the per-path
weight (lhsT = x_m^T [c, n], rhs = W[path] [c, d]) giving out [n, d] in
natural order; reassemble [128, 2048] blocks and store with 1MB DMAs.
"""

import sys

sys.path.insert(0, "/opt/trn_rl_repo")

import numpy as np

# hardcoded problem shape (self-contained; do not read spec/reference)
N = 65536
DIM = 16
C_IN = 128
C_OUT = 128
NUM_PATHS = 4
SEG_IDS = [0, 1, 1, 1, 2, 2, 2, 2, 2, 3, 3, 3, 3, 3, 3, 3]
N_CORES = 8
N_SHARD = N // N_CORES  # 8192 nodes per core
BLOCK = 128             # nodes per SBUF block
N_BLOCKS = N_SHARD // BLOCK  # 64
M_GROUP = 4             # m's per PSUM bank group

_cache = {}


def _build():
    import concourse.bass as bass
    import concourse.mybir as mybir
    import concourse.tile as tile
    from concourse import bacc
    from concourse.masks import make_identity

    f32 = mybir.dt.float32

    nc = bacc.Bacc("TRN2", target_bir_lowering=False, debug=False,
                   num_devices=N_CORES)
    x_d = nc.dram_tensor("x", [N_SHARD, DIM, C_IN], f32, kind="ExternalInput")
    w_d = nc.dram_tensor("w", [NUM_PATHS, C_IN, C_OUT], f32,
                         kind="ExternalInput")
    o_d = nc.dram_tensor("out", [N_SHARD, DIM, C_OUT], f32,
                         kind="ExternalOutput")

    with tile.TileContext(nc) as tc:
        with (
            tc.tile_pool(name="const", bufs=1) as const_pool,
            tc.tile_pool(name="xin", bufs=3) as in_pool,
            tc.tile_pool(name="xout", bufs=3) as out_pool,
            tc.tile_pool(name="xt_sb", bufs=4) as xts_pool,
            tc.tile_pool(name="xt_ps", bufs=3, space="PSUM") as xtp_pool,
            tc.tile_pool(name="o_ps", bufs=3, space="PSUM") as outp_pool,
        ):
            ident = const_pool.tile([128, 128], f32)
            make_identity(nc, ident[:])

            # weight in SBUF: [c, path, d]
            w_sb = const_pool.tile([C_IN, NUM_PATHS, C_OUT], f32)
            nc.sync.dma_start(w_sb[:], w_d.ap().rearrange("p c d -> c p d"))

            for b in range(N_BLOCKS):
                in_t = in_pool.tile([BLOCK, DIM, C_IN], f32)
                nc.sync.dma_start(in_t[:], x_d.ap()[b * BLOCK:(b + 1) * BLOCK])
                out_t = out_pool.tile([BLOCK, DIM, C_OUT], f32)

                for g in range(DIM // M_GROUP):
                    xt_ps = xtp_pool.tile([C_IN, M_GROUP * BLOCK], f32)
                    for j in range(M_GROUP):
                        m = g * M_GROUP + j
                        nc.tensor.transpose(
                            xt_ps[:, j * BLOCK:(j + 1) * BLOCK],
                            in_t[:, m, :],
                            ident[:],
                        )
                    xt_sb = xts_pool.tile([C_IN, M_GROUP * BLOCK], f32)
                    nc.vector.tensor_copy(xt_sb[:], xt_ps[:])

                    o_ps = outp_pool.tile([BLOCK, M_GROUP * C_OUT], f32)
                    for j in range(M_GROUP):
                        m = g * M_GROUP + j
                        nc.tensor.matmul(
                            o_ps[:, j * C_OUT:(j + 1) * C_OUT],
                            lhsT=xt_sb[:, j * BLOCK:(j + 1) * BLOCK],
                            rhs=w_sb[:, SEG_IDS[m], :],
                            start=True, stop=True,
                        )
                    nc.scalar.copy(
                        out=out_t[:, g * M_GROUP:(g + 1) * M_GROUP, :],
                        in_=o_ps[:],
                    )

                nc.scalar.dma_start(o_d.ap()[b * BLOCK:(b + 1) * BLOCK],
                                    out_t[:])

    nc.compile()
    return nc


def _get_nc():
    if "nc" not in _cache:
        _cache["nc"] = _build()
    return _cache["nc"]


def _run(x, weight, trace=False, **trace_kw):
    from concourse.bass_utils import run_bass_kernel_spmd

    nc = _get_nc()
    x = np.ascontiguousarray(x, dtype=np.float32)
    weight = np.ascontiguousarray(weight, dtype=np.float32)
    in_maps = [
        {"x": x[i * N_SHARD:(i + 1) * N_SHARD], "w": weight}
        for i in range(N_CORES)
    ]
    res = run_bass_kernel_spmd(nc, in_maps, list(range(N_CORES)),
                               trace=trace, **trace_kw)
    out = np.concatenate([res.results[i]["out"] for i in range(N_CORES)],
                         axis=0)
    return out, res


def kernel(x, weight):
    out, _ = _run(x, weight, trace=False)
    return out


if __name__ == "__main__":
    rng = np.random.default_rng(0)
    x = rng.standard_normal((N, DIM, C_IN), dtype=np.float32)
    w = rng.standard_normal((NUM_PATHS, C_IN, C_OUT), dtype=np.float32)
    w /= np.sqrt(C_IN)
    out = kernel(x, w)
    w_rows = w[SEG_IDS]
    exp = np.einsum("nmc,mcd->nmd", x, w_rows)
    err = np.abs(out - exp).max() / np.abs(exp).max()
    print("rel err:", err)
